# revision 12
# baseline (speedup 1.0000x reference)
"""Trainium2 Bass kernel for ConvAttnPool (v3).

Model (per batch row b):
  e   = W_emb[x[b]]                       # [T=2500, E=100]
  h   = tanh(conv1d(e, conv_w, pad=5))    # [T'=2501, F=50]
  s   = h @ U_w.T                         # [T', Y]
  a   = softmax(s, axis=t)
  y   = sum_f final_w[y,f] * (a.T @ h)[y,f] + final_b[y]

Device strategy: data-parallel over batch (8 cores x 1 row). Per core:
  - indirect-DMA gather of embedding rows -> eT [E+1, t] bf16 (row E = ones
    so conv tap 0 adds conv_b; generalizes the zero bias)
  - conv flipped: out [t-chunk 128, F] per tap (free dim = 50, cheap),
    tanh -> h_ext [128, 20, 52] bf16 ([t, chunk, m]; col 50 = ones for the
    softmax denominator), PE-transpose -> hT [F, 2560] f32r for scores
  - scores per (ytile, chunk): hT-chunk stationary, U slice moving (f32r,
    1 cyc/col) -> PSUM [128t, yw]
  - exp split across engines: chunks 0..11 scalar-engine Exp -> bf16;
    chunks 12..19 DVE Schraudolph (s*A16+B16 -> int16, bitcast bf16)
  - pooling flipped: stationary exps [t, 128y-slice], moving h_ext chunk
    [t, 52] -> acc [128y, 4*52] PSUM accumulated over chunks (51 cyc/instr)
  - finalize: p = acc * fw_slice (DVE), numer = reduce_X(p), den = acc col 50
  - epilogue: y = numer / den + fb, output [128, 72] y-major slices
"""

import sys

import numpy as np

if "/opt/trn_rl_repo" not in sys.path:
    sys.path.insert(0, "/opt/trn_rl_repo")

import concourse.bass as bass
import concourse.tile as tile
from concourse import bacc, mybir
from concourse.bass_utils import run_bass_kernel_spmd
from concourse.masks import make_identity

VOCAB, E, F, KW, Y = 51917, 100, 50, 10, 8921
B, T = 8, 2500
PAD = 5
TP = T + 1            # conv output length 2501
TC = 20               # t-chunks of 128 (covers 2560)
TPADDED = TC * 128
GLAST = T - 19 * 128  # valid gather rows in last chunk = 68
TLAST = TP - 19 * 128  # valid conv rows in last chunk = 69
ET_W = 2570           # eT width (2560 + 9 taps, zero tail)
EC = E + 1            # conv contraction rows incl. ones/bias row
YW_FULL = 512
NYT = 18              # 17 full y tiles + one 256-wide tail (8960 >= 8921)
YW_LAST = 256
NSLOT = 72            # 18 * 4 y-slice slots of 128 labels
MW = 52               # pool free width: 50 h + ones col + pad (8B aligned)
NACT = 12             # chunks 0..NACT-1 use scalar-engine exp, rest DVE

A16 = 128.0 / float(np.log(2.0))       # schraudolph scale (bf16 bitcast)
B16 = 127.0 * 128.0 - 5.5              # offset incl. tuned adjustment

DEBUG = False

FP32 = mybir.dt.float32
F32R = mybir.dt.float32r
BF16 = mybir.dt.bfloat16
I16 = mybir.dt.int16
I32 = mybir.dt.int32

EXP_GROUPS = [(0, 3), (3, 3), (6, 3), (9, 3), (12, 3), (15, 3), (18, 2)]


def build_program():
    nc = bacc.Bacc(
        "TRN2",
        target_bir_lowering=False,
        debug=False,
        num_devices=B,
    )

    x_d = nc.dram_tensor("x_idx", [128, TC], I32, kind="ExternalInput")
    emb_d = nc.dram_tensor("w_emb", [VOCAB, E], FP32, kind="ExternalInput")
    cwt_d = nc.dram_tensor("cwt", [EC, KW * F], BF16, kind="ExternalInput")
    us_d = nc.dram_tensor("us", [F, NSLOT * 128], F32R, kind="ExternalInput")
    fw_d = nc.dram_tensor("fw", [128, NSLOT * MW], FP32, kind="ExternalInput")
    fbf_d = nc.dram_tensor("fbf", [128, NSLOT], FP32, kind="ExternalInput")
    y_d = nc.dram_tensor("y", [128, NSLOT], FP32, kind="ExternalOutput")
    if DEBUG:
        nd_d = nc.dram_tensor("nd_dbg", [128, 2 * NSLOT], FP32,
                              kind="ExternalOutput")
        ex_d = nc.dram_tensor("ex_dbg", [128, TC * YW_FULL], BF16,
                              kind="ExternalOutput")

    EXP = mybir.ActivationFunctionType.Exp
    TANH = mybir.ActivationFunctionType.Tanh
    MULT = mybir.AluOpType.mult
    ADD = mybir.AluOpType.add

    with tile.TileContext(nc) as tc:
        with tc.tile_pool(name="singles", bufs=1) as singles:
            identity = singles.tile([128, 128], FP32)
            make_identity(nc, identity[:])
            identity_bf = singles.tile([128, 128], BF16)
            make_identity(nc, identity_bf[:])

            x_sb = singles.tile([128, TC], I32)
            nc.sync.dma_start(x_sb[:], x_d[:])
            cwt_sb = singles.tile([EC, KW * F], BF16)
            nc.sync.dma_start(cwt_sb[:], cwt_d[:])
            fbf_sb = singles.tile([128, NSLOT], FP32)
            nc.sync.dma_start(fbf_sb[:], fbf_d[:])

            zeros2 = singles.tile([1, 256], BF16)
            nc.gpsimd.memset(zeros2[:], 0.0)
            eT = singles.tile([EC, ET_W], BF16)
            # engine APs need 32-aligned partition bases: build the ones row
            # (partition 100) via layered memsets from base 96
            nc.gpsimd.memset(eT[96:EC, :], 1.0)
            nc.gpsimd.memset(eT[96:E, :], 0.0)
            nc.gpsimd.memset(eT[0:96, :], 0.0)
            h_ext = singles.tile([128, TC, MW], BF16)
            nc.gpsimd.memset(h_ext[:, :, :], 0.0)
            hT = singles.tile([F, TPADDED], F32R)
            numer_all = singles.tile([128, NSLOT], FP32)
            den_all = singles.tile([128, NSLOT], FP32)

            # ---------------- setup: gather + conv + h forms ----------------
            with (
                tc.tile_pool(name="gat", bufs=3) as gat,
                tc.tile_pool(name="ps_g", bufs=2, space="PSUM") as ps_g,
                tc.tile_pool(name="ps_c", bufs=2, space="PSUM") as ps_c,
                tc.tile_pool(name="ps_t", bufs=2, space="PSUM") as ps_t,
            ):
                for c in range(TC):
                    rows = 128 if c < TC - 1 else GLAST
                    er = gat.tile([128, E], FP32, tag="er")
                    nc.gpsimd.indirect_dma_start(
                        out=er[0:rows, :],
                        out_offset=None,
                        in_=emb_d[:, :],
                        in_offset=bass.IndirectOffsetOnAxis(
                            ap=x_sb[0:rows, c : c + 1], axis=0
                        ),
                    )
                    pt = ps_g.tile([E, 128], FP32, tag="pt")
                    nc.tensor.transpose(
                        pt[:, 0:rows], er[0:rows, :], identity[0:rows, 0:rows]
                    )
                    eslice = eT[0:E, PAD + 128 * c : PAD + 128 * c + rows]
                    if c % 2 == 0:
                        nc.vector.tensor_copy(eslice, pt[:, 0:rows])
                    else:
                        nc.scalar.copy(eslice, pt[:, 0:rows])

                for c in range(TC):
                    tv = 128 if c < TC - 1 else TLAST
                    hc = ps_c.tile([128, F], FP32, tag="hc")
                    for k in range(KW):
                        nc.tensor.matmul(
                            hc[:, :],
                            eT[:, c * 128 + k : c * 128 + k + 128],
                            cwt_sb[:, k * F : (k + 1) * F],
                            start=(k == 0),
                            stop=(k == KW - 1),
                        )
                    nc.scalar.activation(h_ext[0:tv, c, 0:F], hc[0:tv, :], TANH)
                    nc.gpsimd.memset(h_ext[0:tv, c, F : F + 1], 1.0)
                    pst = ps_t.tile([F, 128], BF16, tag="pst")
                    nc.tensor.transpose(
                        pst[:, :], h_ext[:, c, 0:F], identity_bf[:, :]
                    )
                    hslice = hT[:, c * 128 : (c + 1) * 128]
                    if c % 2 == 0:
                        nc.scalar.copy(hslice, pst[:, :])
                    else:
                        nc.vector.tensor_copy(hslice, pst[:, :])

            # ---------------- main loop over y tiles ----------------
            with (
                tc.tile_pool(name="usp", bufs=2) as usp,
                tc.tile_pool(name="fwp", bufs=2) as fwp,
                tc.tile_pool(name="expp", bufs=2) as expp,
                tc.tile_pool(name="ptp", bufs=2) as ptp,
                tc.tile_pool(name="ps_s", bufs=3, space="PSUM") as ps_s,
                tc.tile_pool(name="ps_acc", bufs=2, space="PSUM") as ps_acc,
            ):
                for yt in range(NYT):
                    yw = YW_FULL if yt < NYT - 1 else YW_LAST
                    nsl = yw // 128
                    slot0 = yt * 4

                    us_t = usp.tile([F, YW_FULL], F32R, tag="us")
                    nc.sync.dma_start(
                        us_t[:, 0:yw], us_d[:, slot0 * 128 : slot0 * 128 + yw]
                    )
                    fw_t = fwp.tile([128, 4, MW], FP32, tag="fw")
                    nc.sync.dma_start(
                        fw_t[:, 0:nsl, :],
                        fw_d[:, slot0 * MW : (slot0 + nsl) * MW],
                    )

                    exps = expp.tile([128, TC, YW_FULL], BF16, tag="exps")
                    acc = ps_acc.tile([128, 4, MW], FP32, tag="acc")
                    # start=True clears has_written for the WHOLE bank, so the
                    # 4 slice accumulators sharing this bank must be armed by
                    # one zero matmul spanning all of them; the pool matmuls
                    # then accumulate with start=False.
                    nc.tensor.matmul(
                        acc[:, 0:4, :],
                        zeros2[0:1, 0:128],
                        zeros2[0:1, 0 : 4 * MW],
                        start=True,
                        stop=False,
                        skip_group_check=True,
                    )

                    def emit_pool(c0):
                        for c in (c0, c0 + 1):
                            for sl in range(nsl):
                                nc.tensor.matmul(
                                    acc[:, sl, :],
                                    exps[:, c, sl * 128 : (sl + 1) * 128],
                                    h_ext[:, c, :],
                                    start=False,
                                    stop=(c == TC - 1),
                                    skip_group_check=True,
                                )

                    # 10 groups of 2 chunks; pool(g) emitted after scores(g+2)
                    # so the in-order PE queue never blocks on exp(g)
                    for g in range(TC // 2):
                        c0 = 2 * g
                        ps = ps_s.tile([128, 2, YW_FULL], FP32, tag="s")
                        for i in range(2):
                            c = c0 + i
                            nc.tensor.matmul(
                                ps[:, i, 0:yw],
                                hT[:, c * 128 : (c + 1) * 128],
                                us_t[:, 0:yw],
                                start=True,
                                stop=True,
                            )
                        if c0 < NACT:
                            nc.scalar.activation(
                                exps[:, c0 : c0 + 2, 0:yw],
                                ps[:, 0:2, 0:yw],
                                EXP,
                            )
                        else:
                            nc.vector.tensor_scalar(
                                exps[:, c0 : c0 + 2, 0:yw].bitcast(I16),
                                ps[:, 0:2, 0:yw],
                                A16,
                                B16,
                                MULT,
                                ADD,
                            )
                        if g >= 2:
                            emit_pool(2 * (g - 2))
                    emit_pool(2 * (TC // 2 - 2))
                    emit_pool(2 * (TC // 2 - 1))
                    if DEBUG and yt == 0:
                        nc.sync.dma_start(ex_d[:, :], exps[:, :, :])

                    p_t = ptp.tile([128, 4, MW], BF16, tag="p")
                    nc.vector.tensor_mul(
                        p_t[:, 0:nsl, :], acc[:, 0:nsl, :], fw_t[:, 0:nsl, :]
                    )
                    nc.vector.tensor_reduce(
                        numer_all[:, slot0 : slot0 + nsl],
                        p_t[:, 0:nsl, :],
                        mybir.AxisListType.X,
                        ADD,
                    )
                    nc.vector.tensor_copy(
                        den_all[:, slot0 : slot0 + nsl], acc[:, 0:nsl, F : F + 1]
                    )

                # epilogue: y = numer / den + fb
                if DEBUG:
                    nc.sync.dma_start(nd_d[:, 0:NSLOT], numer_all[:])
                    nc.sync.dma_start(nd_d[:, NSLOT : 2 * NSLOT], den_all[:])
                recip = singles.tile([128, NSLOT], FP32)
                nc.vector.reciprocal(recip[:], den_all[:])
                yv = singles.tile([128, NSLOT], FP32)
                nc.vector.tensor_mul(yv[:], numer_all[:], recip[:])
                yout = singles.tile([128, NSLOT], FP32)
                nc.vector.tensor_add(yout[:], yv[:], fbf_sb[:])
                nc.sync.dma_start(y_d[:, :], yout[:])

    nc.compile()
    return nc


_CACHE = {}


def get_program():
    if "nc" not in _CACHE:
        _CACHE["nc"] = build_program()
    return _CACHE["nc"]


def make_in_maps(x, W_emb, conv_w, conv_b, U_w, final_w, final_b):
    bf = mybir.dt.np(BF16)
    x = np.asarray(x).astype(np.int32)
    x_pad = np.zeros((B, TPADDED), np.int32)
    x_pad[:, :T] = x
    # x_maps[b][p, c] = x[b, c*128 + p]
    x_maps = np.ascontiguousarray(x_pad.reshape(B, TC, 128).transpose(0, 2, 1))

    emb = np.ascontiguousarray(np.asarray(W_emb, np.float32))

    # cwt[e, k*F + f] = conv_w[f, e, k]; row E: conv_b on tap 0 only
    cwt = np.zeros((EC, KW * F), np.float32)
    cwt[0:E] = np.asarray(conv_w, np.float32).transpose(1, 2, 0).reshape(E, KW * F)
    cwt[E, 0:F] = np.asarray(conv_b, np.float32)
    cwt = cwt.astype(bf)

    us = np.zeros((F, NSLOT * 128), np.float32)
    us[:, :Y] = np.asarray(U_w, np.float32).T

    fw = np.zeros((128, NSLOT, MW), np.float32)
    fw_src = np.zeros((NSLOT * 128, F), np.float32)
    fw_src[:Y] = np.asarray(final_w, np.float32)
    fw[:, :, 0:F] = fw_src.reshape(NSLOT, 128, F).transpose(1, 0, 2)
    fw = np.ascontiguousarray(fw.reshape(128, NSLOT * MW))

    fbf = np.zeros((128, NSLOT), np.float32)
    fb_src = np.zeros(NSLOT * 128, np.float32)
    fb_src[:Y] = np.asarray(final_b, np.float32)
    fbf[:, :] = fb_src.reshape(NSLOT, 128).T

    return [
        dict(x_idx=x_maps[b], w_emb=emb, cwt=cwt, us=us, fw=fw, fbf=fbf)
        for b in range(B)
    ]


def run(in_maps, trace=False, **kwargs):
    nc = get_program()
    return run_bass_kernel_spmd(
        nc, in_maps, core_ids=list(range(B)), trace=trace, **kwargs
    )


def _unpack(res):
    # y[b, 128*slot + p] = out[p, slot]
    return np.stack(
        [
            np.asarray(res.results[b]["y"], np.float32).T.reshape(-1)[:Y]
            for b in range(B)
        ]
    )


def kernel(x, W_emb, conv_w, conv_b, U_w, final_w, final_b):
    in_maps = make_in_maps(x, W_emb, conv_w, conv_b, U_w, final_w, final_b)
    res = run(in_maps)
    return _unpack(res)
